# revision 27
# baseline (speedup 1.0000x reference)
"""Bass/Trainium2 kernel for attention-energy softmax:
  proj = enc @ W.T + b        [S,B,D]
  energies[b,s] = hidden[b] . proj[s,b]
  out = softmax(energies, axis=s)[:, None, :]

Algebraic fusion: energies[b,s] = (hidden[b] @ W) . enc[s,b] + hidden[b].b
The bias term is constant per b and cancels in softmax, so it is dropped.
v = hidden @ W is a tiny [B,D]x[D,D] matmul; the kernel then streams the
512MB encoder_outputs once (memory bound), data-parallel over B on 8 cores.

Per-core pipeline:
  1. [W|hidden.T] rows arrive permuted (row = 2p+j per quarter-group) in
     4 chunked DMAs; fp32 v matmuls accumulate over the permuted
     contraction chunks as each group lands, on a pre-warmed PE.
  2. v is broadcast to all 128 partitions with fp32 selector matmuls
     (no HBM traffic) into 2-bank PSUM tiles; each b drains with one
     [128,1024] copy (b=0 on the still-idle vector engine so the first
     multiply starts early, the rest on the scalar engine).
  3. enc streams as 4MB tiles of 32KB partition lines; the first and
     last tiles arrive as four 1MB b-pair sub-DMAs so DVE starts as
     soon as [wx | first pair] lands and drains quickly at the end.
     Each tile feeds fused multiply+reduce (scalar_tensor_tensor with
     accum) ops on DVE.
  4. Softmax per 4-batch group in a transposed [(b*16+t), p] layout:
     one PE transpose, one fused exp+sum activation, and 0/1-indicator
     matmuls for segment max/sum broadcasts; output DMAs straight from
     the transposed tile.
"""

import numpy as np

import concourse.bass as bass
import concourse.mybir as mybir
from concourse import bacc
from concourse.masks import make_identity
from concourse.bass_utils import run_bass_kernel_spmd
from concourse.tile import TileContext

S, B, D = 2048, 64, 1024
NCORES = 8
BL = B // NCORES  # 8 local batches per core
P = 128
T = S // P  # 16 seq tiles
GW = BL // 2  # softmax group width
F32 = mybir.dt.float32

TRACE = False  # test.py sets True to profile

_CACHE = {}

# softmax segment indicators: ind4[j, q] = indT[q, j] = 1 iff q // 16 == j
_INDT = np.repeat(np.eye(GW, dtype=np.float32), T, axis=0)
_IND4 = np.ascontiguousarray(_INDT.T)


def build_kernel() -> bass.Bass:
    nc = bacc.Bacc(None, target_bir_lowering=False)
    enc = nc.dram_tensor("enc", [S, BL, D], F32, kind="ExternalInput")
    # [W|hidden.T] with rows permuted: group g, partition p, sub-chunk j
    # holds original row 256g + 2p + j (8.25KB contiguous per partition).
    wx = nc.dram_tensor("wx", [4, P, 2, D + BL], F32, kind="ExternalInput")
    ind4_d = nc.dram_tensor("ind4", [GW, GW * T], F32, kind="ExternalInput")
    ind4n_d = nc.dram_tensor("ind4n", [GW, GW * T], F32, kind="ExternalInput")
    indT_d = nc.dram_tensor("indT", [GW * T, GW], F32, kind="ExternalInput")
    out = nc.dram_tensor("out", [BL, S], F32, kind="ExternalOutput")
    DB = D + BL

    with TileContext(nc) as tc:
        with (
            tc.tile_pool(name="consts", bufs=1) as consts,
            tc.tile_pool(name="work", bufs=3) as work,
            tc.tile_pool(name="small", bufs=2) as small,
            tc.tile_pool(name="mm", bufs=2, space="PSUM") as mmp,
            tc.tile_pool(name="vbp", bufs=2, space="PSUM") as vbp,
            tc.tile_pool(name="ptr", bufs=1, space="PSUM") as ptr,
            tc.tile_pool(name="pstat", bufs=1, space="PSUM") as pstat,
        ):
            ident = consts.tile([P, P], F32)
            make_identity(nc, ident)
            # Warm the PE p-state (needs ~3us of continuous work to reach
            # 2.4GHz) while the weight DMAs are in flight, so the v matmuls
            # run at full clock.
            warm_ps = mmp.tile([P, P], F32, tag="mm")
            for _ in range(8):
                nc.tensor.matmul(warm_ps, ident, ident, start=True, stop=True)

            # ---- chunked load of [W|hT]; v matmuls overlap the DMAs ----
            wx_sb = []
            for g in range(4):
                wt = consts.tile([P, 2, DB], F32, tag=f"wx{g}")
                nc.sync.dma_start(out=wt, in_=wx[g])
                wx_sb.append(wt)

            # selector tiles: sel[k, b, m] = 1 if k == b else 0
            ones8 = consts.tile([BL, P], F32)
            nc.vector.memset(ones8, 1.0)
            sel = consts.tile([BL, BL, P], F32)
            for b in range(BL):
                nc.vector.tensor_scalar_mul(
                    sel[:, b, :], ones8, ident[0:BL, b : b + 1]
                )
            ind4 = consts.tile([GW, GW * T], F32)
            nc.sync.dma_start(out=ind4, in_=ind4_d[:, :])
            ind4n = consts.tile([GW, GW * T], F32)
            nc.sync.dma_start(out=ind4n, in_=ind4n_d[:, :])
            indT64 = consts.tile([GW * T, GW], F32)
            nc.sync.dma_start(out=indT64, in_=indT_d[:, :])

            # v = hidden_local @ W -> [BL, D] (all fp32 for accuracy).
            # Contraction chunk (g, j) covers rows {256g + 2p + j}; the
            # union over 8 chunks is each row exactly once, so the PSUM
            # accumulation is exact regardless of order.
            v_sb = consts.tile([BL, D], F32)
            v_ps0 = mmp.tile([BL, 512], F32, tag="mm")
            v_ps1 = mmp.tile([BL, 512], F32, tag="mm")
            for g in range(4):
                for j in range(2):
                    for half, v_ps in ((0, v_ps0), (1, v_ps1)):
                        nc.tensor.matmul(
                            v_ps,
                            wx_sb[g][:, j, D : D + BL],
                            wx_sb[g][:, j, half * 512 : (half + 1) * 512],
                            start=(g == 0 and j == 0),
                            stop=(g == 3 and j == 1),
                        )
            nc.scalar.copy(out=v_sb[:, 0:512], in_=v_ps0)
            nc.scalar.copy(out=v_sb[:, 512:1024], in_=v_ps1)

            # ---- broadcast v to all partitions: vb[p, b, d] = v[b, d] ----
            # Both 512-col halves land in one 2-bank PSUM tile so each b
            # drains with a single [P, 1024] copy; 2 rotating PSUM tiles
            # decouple the PE from the copy engine.
            vb = consts.tile([P, BL, D], F32)
            for b in range(BL):
                bc_ps = vbp.tile([P, D], F32, tag="vb")
                for h in range(2):
                    nc.tensor.matmul(
                        bc_ps[:, h * 512 : (h + 1) * 512],
                        sel[:, b, :],
                        v_sb[:, h * 512 : (h + 1) * 512],
                        start=True,
                        stop=True,
                    )
                # b=0 copies on the (still idle) vector engine so the
                # first multiply can start ASAP; the rest stream on ACT
                eng = nc.vector.tensor_copy if b == 0 else nc.scalar.copy
                eng(out=vb[:, b, :], in_=bc_ps)

            # ---- energies: e_all[p, b, t] = sum_d enc[t*128+p, b, d]*v[b, d] ----
            e_all = consts.tile([P, BL, T], F32)
            dummy = consts.tile([P, 1], F32)
            out_r = out[:, :].rearrange("b (t p) -> (b t) p", p=P)

            def stt(src, j, b, t):
                # fused multiply + free-dim sum in one DVE pass:
                # out = (in0 * 1.0) * in1, accum = sum(out)
                nc.vector.scalar_tensor_tensor(
                    out=dummy.broadcast_to((P, D)),
                    in0=src[:, j, :],
                    scalar=1.0,
                    in1=vb[:, b, :],
                    op0=mybir.AluOpType.mult,
                    op1=mybir.AluOpType.mult,
                    accum_out=e_all[:, b, t : t + 1],
                )

            def tile_dma(t, et, pairs):
                if pairs:
                    for q in range(4):
                        nc.sync.dma_start(
                            out=et[:, 2 * q : 2 * q + 2, :],
                            in_=enc[t * P : (t + 1) * P, 2 * q : 2 * q + 2, :],
                        )
                else:
                    # 113+15 line split: descriptor round-robin restarts at
                    # ring 0 per dma_start, so the (persistently slower)
                    # ring 15 gets 7 lines per tile instead of 8
                    nc.sync.dma_start(
                        out=et[0:113], in_=enc[t * P : t * P + 113, :, :]
                    )
                    nc.sync.dma_start(
                        out=et[113:P], in_=enc[t * P + 113 : (t + 1) * P, :, :]
                    )

            for t in range(T - 1):
                et = work.tile([P, BL, D], F32, tag="enc_t")
                tile_dma(t, et, pairs=(t == 0))
                for j in range(BL):
                    stt(et, j, j, t)

            # last seq tile in 4 b-pair sub-DMAs so the DVE drain after
            # the final HBM byte is short and group-0 softmax overlaps.
            t = T - 1
            el = work.tile([P, BL, D], F32, tag="enc_t")
            tile_dma(t, el, pairs=True)

            def softmax_group(g):
                """Softmax for b in [4g, 4g+4) in transposed layout."""
                g0 = GW * g
                # group max: per-(p,b) max over t, cross partitions via PE
                m8g = small.tile([P, GW], F32, tag=f"m8{g}")
                nc.vector.tensor_reduce(
                    out=m8g,
                    in_=e_all[:, g0 : g0 + GW, :],
                    axis=mybir.AxisListType.X,
                    op=mybir.AluOpType.max,
                )
                trm = pstat.tile([GW, P], F32, tag="stat")
                nc.tensor.transpose(trm, m8g, ident)
                gmax = small.tile([GW, 1], F32, tag=f"gmax{g}")
                nc.vector.tensor_reduce(
                    out=gmax, in_=trm, axis=mybir.AxisListType.X,
                    op=mybir.AluOpType.max,
                )
                negP_ps = pstat.tile([GW * T, 1], F32, tag="stat")
                nc.tensor.matmul(negP_ps, ind4n, gmax, start=True, stop=True)
                negP = small.tile([GW * T, 1], F32, tag=f"negP{g}")
                nc.scalar.copy(out=negP, in_=negP_ps)
                # transpose energies to [(b-g0)*16+t, p] and exp+sum
                eT_ps = ptr.tile([GW * T, P], F32, tag="tr")
                nc.tensor.transpose(eT_ps, e_all[:, g0 : g0 + GW, :], ident)
                eTg = small.tile([GW * T, P], F32, tag=f"eT{g}")
                s1g = small.tile([GW * T, 1], F32, tag=f"s1{g}")
                nc.scalar.activation(
                    out=eTg,
                    in_=eT_ps,
                    func=mybir.ActivationFunctionType.Exp,
                    bias=negP,
                    accum_out=s1g,
                )
                # per-b sums, reciprocal, broadcast back per partition
                sums_ps = pstat.tile([GW, 1], F32, tag="stat")
                nc.tensor.matmul(sums_ps, indT64, s1g, start=True, stop=True)
                recipg = small.tile([GW, 1], F32, tag=f"recip{g}")
                nc.vector.reciprocal(recipg, sums_ps)
                rP_ps = pstat.tile([GW * T, 1], F32, tag="stat")
                nc.tensor.matmul(rP_ps, ind4, recipg, start=True, stop=True)
                rP = small.tile([GW * T, 1], F32, tag=f"rP{g}")
                nc.scalar.copy(out=rP, in_=rP_ps)
                outTg = small.tile([GW * T, P], F32, tag=f"oT{g}")
                nc.vector.tensor_scalar_mul(outTg, eTg, rP[:, 0:1])
                nc.sync.dma_start(
                    out=out_r[GW * T * g : GW * T * (g + 1), :], in_=outTg
                )

            for q in range(4):
                stt(el, 2 * q, 2 * q, t)
                stt(el, 2 * q + 1, 2 * q + 1, t)
                if q == 1:
                    softmax_group(0)
            softmax_group(1)

    nc.compile()
    return nc


def kernel(hidden, encoder_outputs, W_attn, b_attn):
    hidden = np.asarray(hidden, dtype=np.float32)
    encoder_outputs = np.asarray(encoder_outputs, dtype=np.float32)
    W_attn = np.asarray(W_attn, dtype=np.float32)

    in_maps = []
    for c in range(NCORES):
        bs = slice(c * BL, (c + 1) * BL)
        wxc = np.concatenate([W_attn, hidden[0, bs, :].T], axis=1)  # [D, D+BL]
        # row permutation: [4 groups, 128 partitions, 2 sub-chunks, D+BL]
        wxp = np.ascontiguousarray(wxc.reshape(4, P, 2, D + BL))
        in_maps.append(
            {
                "enc": np.ascontiguousarray(encoder_outputs[:, bs, :]),
                "wx": wxp,
                "ind4": _IND4,
                "ind4n": -_IND4,
                "indT": _INDT,
            }
        )

    if "nc" not in _CACHE:
        _CACHE["nc"] = build_kernel()
    nc = _CACHE["nc"]

    res = run_bass_kernel_spmd(nc, in_maps, core_ids=list(range(NCORES)), trace=TRACE)
    if TRACE:
        _CACHE["last_result"] = res
    out = np.concatenate([r["out"] for r in res.results], axis=0)  # [B, S]
    return out[:, None, :]


# revision 28
# speedup vs baseline: 8.7028x; 8.7028x over previous
"""Bass/Trainium2 kernel for attention-energy softmax:
  proj = enc @ W.T + b        [S,B,D]
  energies[b,s] = hidden[b] . proj[s,b]
  out = softmax(energies, axis=s)[:, None, :]

Algebraic fusion: energies[b,s] = (hidden[b] @ W) . enc[s,b] + hidden[b].b
The bias term is constant per b and cancels in softmax, so it is dropped.
v = hidden @ W is a tiny [B,D]x[D,D] matmul; the kernel then streams the
512MB encoder_outputs once (memory bound), data-parallel over B on 8 cores.

Per-core pipeline:
  1. [W|hidden.T] rows arrive permuted (row = 2p+j per quarter-group) in
     4 chunked DMAs; fp32 v matmuls accumulate over the permuted
     contraction chunks as each group lands, on a pre-warmed PE.
  2. v is broadcast to all 128 partitions with fp32 selector matmuls
     (no HBM traffic) into 2-bank PSUM tiles; each b drains with one
     [128,1024] copy (b=0 on the still-idle vector engine so the first
     multiply starts early, the rest on the scalar engine).
  3. enc streams as 4MB tiles of 32KB partition lines; the first and
     last tiles arrive as four 1MB b-pair sub-DMAs so DVE starts as
     soon as [wx | first pair] lands and drains quickly at the end.
     Each tile feeds fused multiply+reduce (scalar_tensor_tensor with
     accum) ops on DVE.
  4. Softmax per 4-batch group in a transposed [(b*16+t), p] layout:
     one PE transpose, one fused exp+sum activation, and 0/1-indicator
     matmuls for segment max/sum broadcasts; output DMAs straight from
     the transposed tile.
"""

import numpy as np

import concourse.bass as bass
import concourse.mybir as mybir
from concourse import bacc
from concourse.masks import make_identity
from concourse.bass_utils import run_bass_kernel_spmd
from concourse.tile import TileContext

S, B, D = 2048, 64, 1024
NCORES = 8
BL = B // NCORES  # 8 local batches per core
P = 128
T = S // P  # 16 seq tiles
GW = BL // 2  # softmax group width
F32 = mybir.dt.float32

TRACE = False  # test.py sets True to profile

_CACHE = {}

# softmax segment indicators: ind4[j, q] = indT[q, j] = 1 iff q // 16 == j
_INDT = np.repeat(np.eye(GW, dtype=np.float32), T, axis=0)
_IND4 = np.ascontiguousarray(_INDT.T)


def build_kernel() -> bass.Bass:
    nc = bacc.Bacc(None, target_bir_lowering=False)
    enc = nc.dram_tensor("enc", [S, BL, D], F32, kind="ExternalInput")
    # [W|hidden.T] with rows permuted: group g, partition p, sub-chunk j
    # holds original row 256g + 2p + j (8.25KB contiguous per partition).
    wx = nc.dram_tensor("wx", [4, P, 2, D + BL], F32, kind="ExternalInput")
    ind4_d = nc.dram_tensor("ind4", [GW, GW * T], F32, kind="ExternalInput")
    ind4n_d = nc.dram_tensor("ind4n", [GW, GW * T], F32, kind="ExternalInput")
    indT_d = nc.dram_tensor("indT", [GW * T, GW], F32, kind="ExternalInput")
    out = nc.dram_tensor("out", [BL, S], F32, kind="ExternalOutput")
    DB = D + BL

    with TileContext(nc) as tc:
        with (
            tc.tile_pool(name="consts", bufs=1) as consts,
            tc.tile_pool(name="work", bufs=3) as work,
            tc.tile_pool(name="small", bufs=2) as small,
            tc.tile_pool(name="mm", bufs=2, space="PSUM") as mmp,
            tc.tile_pool(name="vbp", bufs=2, space="PSUM") as vbp,
            tc.tile_pool(name="ptr", bufs=1, space="PSUM") as ptr,
            tc.tile_pool(name="pstat", bufs=1, space="PSUM") as pstat,
        ):
            ident = consts.tile([P, P], F32)
            make_identity(nc, ident)
            # Warm the PE p-state (needs ~3us of continuous work to reach
            # 2.4GHz) while the weight DMAs are in flight, so the v matmuls
            # run at full clock.
            warm_ps = mmp.tile([P, P], F32, tag="mm")
            for _ in range(8):
                nc.tensor.matmul(warm_ps, ident, ident, start=True, stop=True)

            # ---- chunked load of [W|hT]; v matmuls overlap the DMAs ----
            wx_sb = []
            for g in range(4):
                wt = consts.tile([P, 2, DB], F32, tag=f"wx{g}")
                nc.sync.dma_start(out=wt, in_=wx[g])
                wx_sb.append(wt)

            # selector tiles: sel[k, b, m] = 1 if k == b else 0
            ones8 = consts.tile([BL, P], F32)
            nc.vector.memset(ones8, 1.0)
            sel = consts.tile([BL, BL, P], F32)
            for b in range(BL):
                nc.vector.tensor_scalar_mul(
                    sel[:, b, :], ones8, ident[0:BL, b : b + 1]
                )
            ind4 = consts.tile([GW, GW * T], F32)
            nc.sync.dma_start(out=ind4, in_=ind4_d[:, :])
            ind4n = consts.tile([GW, GW * T], F32)
            nc.sync.dma_start(out=ind4n, in_=ind4n_d[:, :])
            indT64 = consts.tile([GW * T, GW], F32)
            nc.sync.dma_start(out=indT64, in_=indT_d[:, :])

            # v = hidden_local @ W -> [BL, D] (all fp32 for accuracy).
            # Contraction chunk (g, j) covers rows {256g + 2p + j}; the
            # union over 8 chunks is each row exactly once, so the PSUM
            # accumulation is exact regardless of order.
            v_sb = consts.tile([BL, D], F32)
            v_ps0 = mmp.tile([BL, 512], F32, tag="mm")
            v_ps1 = mmp.tile([BL, 512], F32, tag="mm")
            for g in range(4):
                for j in range(2):
                    for half, v_ps in ((0, v_ps0), (1, v_ps1)):
                        nc.tensor.matmul(
                            v_ps,
                            wx_sb[g][:, j, D : D + BL],
                            wx_sb[g][:, j, half * 512 : (half + 1) * 512],
                            start=(g == 0 and j == 0),
                            stop=(g == 3 and j == 1),
                        )
            nc.scalar.copy(out=v_sb[:, 0:512], in_=v_ps0)
            nc.scalar.copy(out=v_sb[:, 512:1024], in_=v_ps1)

            # ---- broadcast v to all partitions: vb[p, b, d] = v[b, d] ----
            # Both 512-col halves land in one 2-bank PSUM tile so each b
            # drains with a single [P, 1024] copy; 2 rotating PSUM tiles
            # decouple the PE from the copy engine.
            vb = consts.tile([P, BL, D], F32)
            for b in range(BL):
                bc_ps = vbp.tile([P, D], F32, tag="vb")
                for h in range(2):
                    nc.tensor.matmul(
                        bc_ps[:, h * 512 : (h + 1) * 512],
                        sel[:, b, :],
                        v_sb[:, h * 512 : (h + 1) * 512],
                        start=True,
                        stop=True,
                    )
                # b=0 copies on the (still idle) vector engine so the
                # first multiply can start ASAP; the rest stream on ACT
                eng = nc.vector.tensor_copy if b == 0 else nc.scalar.copy
                eng(out=vb[:, b, :], in_=bc_ps)

            # ---- energies: e_all[p, b, t] = sum_d enc[t*128+p, b, d]*v[b, d] ----
            e_all = consts.tile([P, BL, T], F32)
            dummy = consts.tile([P, 1], F32)
            out_r = out[:, :].rearrange("b (t p) -> (b t) p", p=P)

            def stt(src, j, b, t):
                # fused multiply + free-dim sum in one DVE pass:
                # out = (in0 * 1.0) * in1, accum = sum(out)
                nc.vector.scalar_tensor_tensor(
                    out=dummy.broadcast_to((P, D)),
                    in0=src[:, j, :],
                    scalar=1.0,
                    in1=vb[:, b, :],
                    op0=mybir.AluOpType.mult,
                    op1=mybir.AluOpType.mult,
                    accum_out=e_all[:, b, t : t + 1],
                )

            def tile_dma(t, et, pairs):
                if pairs:
                    for q in range(4):
                        nc.sync.dma_start(
                            out=et[:, 2 * q : 2 * q + 2, :],
                            in_=enc[t * P : (t + 1) * P, 2 * q : 2 * q + 2, :],
                        )
                else:
                    nc.sync.dma_start(
                        out=et, in_=enc[t * P : (t + 1) * P, :, :]
                    )

            for t in range(T - 1):
                et = work.tile([P, BL, D], F32, tag="enc_t")
                tile_dma(t, et, pairs=(t == 0))
                for j in range(BL):
                    stt(et, j, j, t)

            # last seq tile in 4 b-pair sub-DMAs so the DVE drain after
            # the final HBM byte is short and group-0 softmax overlaps.
            t = T - 1
            el = work.tile([P, BL, D], F32, tag="enc_t")
            tile_dma(t, el, pairs=True)

            def softmax_group(g):
                """Softmax for b in [4g, 4g+4) in transposed layout."""
                g0 = GW * g
                # group max: per-(p,b) max over t, cross partitions via PE
                m8g = small.tile([P, GW], F32, tag=f"m8{g}")
                nc.vector.tensor_reduce(
                    out=m8g,
                    in_=e_all[:, g0 : g0 + GW, :],
                    axis=mybir.AxisListType.X,
                    op=mybir.AluOpType.max,
                )
                trm = pstat.tile([GW, P], F32, tag="stat")
                nc.tensor.transpose(trm, m8g, ident)
                gmax = small.tile([GW, 1], F32, tag=f"gmax{g}")
                nc.vector.tensor_reduce(
                    out=gmax, in_=trm, axis=mybir.AxisListType.X,
                    op=mybir.AluOpType.max,
                )
                negP_ps = pstat.tile([GW * T, 1], F32, tag="stat")
                nc.tensor.matmul(negP_ps, ind4n, gmax, start=True, stop=True)
                negP = small.tile([GW * T, 1], F32, tag=f"negP{g}")
                nc.scalar.copy(out=negP, in_=negP_ps)
                # transpose energies to [(b-g0)*16+t, p] and exp+sum
                eT_ps = ptr.tile([GW * T, P], F32, tag="tr")
                nc.tensor.transpose(eT_ps, e_all[:, g0 : g0 + GW, :], ident)
                eTg = small.tile([GW * T, P], F32, tag=f"eT{g}")
                s1g = small.tile([GW * T, 1], F32, tag=f"s1{g}")
                nc.scalar.activation(
                    out=eTg,
                    in_=eT_ps,
                    func=mybir.ActivationFunctionType.Exp,
                    bias=negP,
                    accum_out=s1g,
                )
                # per-b sums, reciprocal, broadcast back per partition
                sums_ps = pstat.tile([GW, 1], F32, tag="stat")
                nc.tensor.matmul(sums_ps, indT64, s1g, start=True, stop=True)
                recipg = small.tile([GW, 1], F32, tag=f"recip{g}")
                nc.vector.reciprocal(recipg, sums_ps)
                rP_ps = pstat.tile([GW * T, 1], F32, tag="stat")
                nc.tensor.matmul(rP_ps, ind4, recipg, start=True, stop=True)
                rP = small.tile([GW * T, 1], F32, tag=f"rP{g}")
                nc.scalar.copy(out=rP, in_=rP_ps)
                outTg = small.tile([GW * T, P], F32, tag=f"oT{g}")
                nc.vector.tensor_scalar_mul(outTg, eTg, rP[:, 0:1])
                nc.sync.dma_start(
                    out=out_r[GW * T * g : GW * T * (g + 1), :], in_=outTg
                )

            for q in range(4):
                stt(el, 2 * q, 2 * q, t)
                stt(el, 2 * q + 1, 2 * q + 1, t)
                if q == 1:
                    softmax_group(0)
            softmax_group(1)

    nc.compile()
    return nc


def kernel(hidden, encoder_outputs, W_attn, b_attn):
    hidden = np.asarray(hidden, dtype=np.float32)
    encoder_outputs = np.asarray(encoder_outputs, dtype=np.float32)
    W_attn = np.asarray(W_attn, dtype=np.float32)

    in_maps = []
    for c in range(NCORES):
        bs = slice(c * BL, (c + 1) * BL)
        wxc = np.concatenate([W_attn, hidden[0, bs, :].T], axis=1)  # [D, D+BL]
        # row permutation: [4 groups, 128 partitions, 2 sub-chunks, D+BL]
        wxp = np.ascontiguousarray(wxc.reshape(4, P, 2, D + BL))
        in_maps.append(
            {
                "enc": np.ascontiguousarray(encoder_outputs[:, bs, :]),
                "wx": wxp,
                "ind4": _IND4,
                "ind4n": -_IND4,
                "indT": _INDT,
            }
        )

    if "nc" not in _CACHE:
        _CACHE["nc"] = build_kernel()
    nc = _CACHE["nc"]

    res = run_bass_kernel_spmd(nc, in_maps, core_ids=list(range(NCORES)), trace=TRACE)
    if TRACE:
        _CACHE["last_result"] = res
    out = np.concatenate([r["out"] for r in res.results], axis=0)  # [B, S]
    return out[:, None, :]
